# revision 21
# baseline (speedup 1.0000x reference)
"""PhysioNeuron Trainium2 kernel.

Data-parallel over 8 NeuronCores: batch B=32768 split into 8 shards of 4096
rows, [d,d] weights replicated. Cross-core reductions (hebb sum, metabolism
mean) are done on the host between two device launches:

launch 1 (per core, B_c=4096, d=1024, 32 row-tiles):
  phase 1: slow = x @ Wslow.T (resident in SBUF), e_sum = sum(|slow|, row)
  phase 2: hebbT[:, 0:512] partial = x.T-chunks x slow halves, 8 psum banks
           accumulated over 32 tiles; s_sum = sum(x), s_ssq = sum(x^2)
  MLP:     Linear(4,16)+LN(16)+tanh+Linear(16,3)+sigmoid regulator folded on
           the host into per-k scalar coefficients of u=1023*stress and
           v=1024*excitation (LN(16) variance is a quadratic in (u,v))
  phase 3: hebbT[:, 512:1024] partial
  outputs: met/sen/gate [128,32] each, hebbT partial halves

host:     hebb = sum_c(partials).T / B; rate = mean(met)*0.1;
          W_fast_new = (W_fast + rate*tanh(hebb)) * 0.9995  (exact fp64)

launch 2: fast = x @ Wfn.T and slow recomputed from the same x tiles;
          combined = slow + gate*fast; y = LN(Silu(beta*combined)) using the
          LN row-scale invariance (beta = 0.5 + 2*sensitivity).

All matmuls run as fp32r (TF32-like) with fp32 PSUM accumulation.
"""

import numpy as np

B = 32768
D = 1024
NC_ = 8
BC = B // NC_        # 4096 rows per core
NT = BC // 128       # 32 tiles per core
EPS = 1e-5

_COMPILED = {}


def _bass_mods():
    from concourse import bacc, bass, tile
    from concourse.bass import mybir
    return bacc, bass, tile, mybir


def _build_p1():
    bacc, bass, tile, mybir = _bass_mods()
    f32 = mybir.dt.float32
    f32r = mybir.dt.float32r
    AF = mybir.ActivationFunctionType
    OP = mybir.AluOpType
    AX = mybir.AxisListType
    ts = bass.ts

    nc = bacc.Bacc(
        "TRN2",
        target_bir_lowering=False,
        debug=False,
        enable_asserts=False,
        num_devices=NC_,
    )

    xt_d = nc.dram_tensor("xt", [NT, 128, 8, 128], f32, kind="ExternalInput")
    xn_d = nc.dram_tensor("xn", [NT, 128, D], f32, kind="ExternalInput")
    wst_d = nc.dram_tensor("wst", [128, 8, D], f32, kind="ExternalInput")
    coef_d = nc.dram_tensor("coef", [128, 128], f32, kind="ExternalInput")
    msg_d = nc.dram_tensor("msg", [128, 96], f32, kind="ExternalOutput")
    hb0_d = nc.dram_tensor("hb0", [128, 8, 512], f32, kind="ExternalOutput")
    hb1_d = nc.dram_tensor("hb1", [128, 8, 512], f32, kind="ExternalOutput")

    with tile.TileContext(nc) as tc:
        with tc.tile_pool(name="persist", bufs=1) as pp, \
             tc.tile_pool(name="stats", bufs=1) as sp:

            slow_sb = pp.tile([128, NT, D], f32, tag="slow")
            coef_sb = sp.tile([128, 128], f32, tag="coef")
            s_sum = sp.tile([128, NT], f32, tag="s_sum")
            s_ssq = sp.tile([128, NT], f32, tag="s_ssq")
            e_sum = sp.tile([128, NT], f32, tag="e_sum")
            met_sb = sp.tile([128, NT], f32, tag="met")
            sen_sb = sp.tile([128, NT], f32, tag="sen")
            gate_sb = sp.tile([128, NT], f32, tag="gate")

            nc.sync.dma_start(coef_sb[:], coef_d[:])

            # ------------- phase 1: slow = x @ Wslow.T, e_sum -------------
            with tc.tile_pool(name="ph1w", bufs=1) as wp, \
                 tc.tile_pool(name="ph1x", bufs=3) as xp, \
                 tc.tile_pool(name="ph1p", bufs=4, space=bass.MemorySpace.PSUM) as ps1:
                wst_sb = wp.tile([128, 8, D], f32, tag="wst")
                nc.sync.dma_start(wst_sb[:].bitcast(f32r), wst_d[:].bitcast(f32r))
                for t in range(NT):
                    xt = xp.tile([128, 8, 128], f32, tag="xt")
                    nc.sync.dma_start(xt[:].bitcast(f32r), xt_d[t].bitcast(f32r))
                    for h in range(2):
                        ps = ps1.tile([128, 512], f32, tag="ps")
                        for c in range(8):
                            nc.tensor.matmul(
                                ps[:],
                                xt[:, c, :].bitcast(f32r),
                                wst_sb[:, c, ts(h, 512)].bitcast(f32r),
                                start=(c == 0),
                                stop=(c == 7),
                            )
                        nc.scalar.copy(slow_sb[:, t, ts(h, 512)].bitcast(f32r), ps[:])
                    nc.vector.tensor_reduce(
                        e_sum[:, t:t + 1],
                        slow_sb[:, t, :],
                        axis=AX.X,
                        op=OP.add,
                        apply_absolute_value=True,
                    )

            # --------- phase 2: hebbT[:, 0:512] partial + stats ---------
            with tc.tile_pool(name="ph2p", bufs=1, space=bass.MemorySpace.PSUM) as ps2:
                hb = [ps2.tile([128, 512], f32, tag=f"hb{c}", name=f"hb{c}")
                      for c in range(8)]
                with tc.tile_pool(name="ph2x", bufs=3) as xnp2:
                    for t in range(NT):
                        xn = xnp2.tile([128, D], f32, tag="xn")
                        nc.sync.dma_start(xn[:].bitcast(f32r), xn_d[t].bitcast(f32r))
                        nc.vector.tensor_reduce(
                            s_sum[:, t:t + 1], xn[:], axis=AX.X, op=OP.add
                        )
                        for c in range(8):
                            nc.tensor.matmul(
                                hb[c][:],
                                xn[:, ts(c, 128)].bitcast(f32r),
                                slow_sb[:, t, 0:512].bitcast(f32r),
                                start=(t == 0),
                                stop=(t == NT - 1),
                            )
                        # in-place square; accum gives sum(x^2) per row
                        nc.scalar.activation(
                            xn[:].bitcast(f32r), xn[:], AF.Square,
                            accum_out=s_ssq[:, t:t + 1],
                        )
                with tc.tile_pool(name="ph2f", bufs=2) as hsp:
                    for c in range(8):
                        hs = hsp.tile([128, 512], f32, tag="hs")
                        nc.scalar.copy(hs[:], hb[c][:])
                        nc.sync.dma_start(hb0_d[:, c, :], hs[:])

            # --------- regulator MLP (tiny; overlaps phase 3 PE) ---------
            with tc.tile_pool(name="mlp", bufs=1) as mp:
                C = coef_sb
                ssq2 = mp.tile([128, NT], f32, tag="ssq2")
                nc.vector.tensor_tensor(ssq2[:], s_sum[:], s_sum[:], OP.mult)
                u_t = mp.tile([128, NT], f32, tag="u_t")
                nc.vector.scalar_tensor_tensor(
                    u_t[:], ssq2[:], -1.0 / D, s_ssq[:],
                    op0=OP.mult, op1=OP.add,
                )
                t1 = mp.tile([128, NT], f32, tag="t1")
                nc.vector.tensor_tensor(t1[:], u_t[:], u_t[:], OP.mult)
                t2 = mp.tile([128, NT], f32, tag="t2")
                nc.vector.tensor_tensor(t2[:], e_sum[:], e_sum[:], OP.mult)
                t3 = mp.tile([128, NT], f32, tag="t3")
                nc.vector.tensor_tensor(t3[:], u_t[:], e_sum[:], OP.mult)
                vacc = mp.tile([128, NT], f32, tag="vacc")
                nc.vector.tensor_scalar_mul(vacc[:], t1[:], C[:, 115:116])
                for src, qi in ((t2, 116), (t3, 117), (u_t, 118), (e_sum, 119)):
                    nc.vector.scalar_tensor_tensor(
                        vacc[:], src[:], C[:, qi:qi + 1], vacc[:],
                        op0=OP.mult, op1=OP.add,
                    )
                # C[:,120] already includes the LN(16) eps (host-folded)
                nc.vector.tensor_scalar_add(vacc[:], vacc[:], C[:, 120:121])
                sd16 = mp.tile([128, NT], f32, tag="sd16")
                nc.scalar.activation(sd16[:], vacc[:], AF.Sqrt, bias=0.0)
                rstd16 = mp.tile([128, NT], f32, tag="rstd16")
                nc.vector.reciprocal(rstd16[:], sd16[:])
                tks = mp.tile([128, 16, NT], f32, tag="tks")
                for k in range(16):
                    m1 = mp.tile([128, NT], f32, tag="m1", bufs=2)
                    nc.vector.tensor_scalar(
                        m1[:], u_t[:],
                        C[:, k:k + 1], C[:, 32 + k:33 + k],
                        op0=OP.mult, op1=OP.add,
                    )
                    m2 = mp.tile([128, NT], f32, tag="m2", bufs=2)
                    nc.vector.scalar_tensor_tensor(
                        m2[:], e_sum[:], C[:, 16 + k:17 + k], m1[:],
                        op0=OP.mult, op1=OP.add,
                    )
                    m3 = mp.tile([128, NT], f32, tag="m3", bufs=2)
                    nc.vector.tensor_tensor(m3[:], m2[:], rstd16[:], OP.mult)
                    nc.scalar.activation(
                        tks[:, k, :], m3[:], AF.Tanh,
                        bias=C[:, 48 + k:49 + k],
                    )
                for m, dst in enumerate((met_sb, sen_sb, gate_sb)):
                    acc = mp.tile([128, NT], f32, tag=f"acc{m}", name=f"acc{m}")
                    nc.vector.tensor_scalar_mul(
                        acc[:], tks[:, 0, :], C[:, 64 + 16 * m:65 + 16 * m]
                    )
                    for k in range(1, 16):
                        nc.vector.scalar_tensor_tensor(
                            acc[:], tks[:, k, :],
                            C[:, 64 + 16 * m + k:65 + 16 * m + k], acc[:],
                            op0=OP.mult, op1=OP.add,
                        )
                    nc.scalar.activation(
                        dst[:], acc[:], AF.Sigmoid,
                        bias=C[:, 112 + m:113 + m],
                    )
            nc.sync.dma_start(msg_d[:, 0:32], met_sb[:])
            nc.sync.dma_start(msg_d[:, 32:64], sen_sb[:])
            nc.sync.dma_start(msg_d[:, 64:96], gate_sb[:])

            # --------- phase 3: hebbT[:, 512:1024] partial ---------
            with tc.tile_pool(name="ph3p", bufs=1, space=bass.MemorySpace.PSUM) as ps3:
                hb2 = [ps3.tile([128, 512], f32, tag=f"hc{c}", name=f"hc{c}")
                       for c in range(8)]
                with tc.tile_pool(name="ph3x", bufs=3) as xnp3:
                    for t in range(NT):
                        xn3 = xnp3.tile([128, D], f32, tag="xn3")
                        nc.sync.dma_start(xn3[:].bitcast(f32r), xn_d[t].bitcast(f32r))
                        for c in range(8):
                            nc.tensor.matmul(
                                hb2[c][:],
                                xn3[:, ts(c, 128)].bitcast(f32r),
                                slow_sb[:, t, 512:1024].bitcast(f32r),
                                start=(t == 0),
                                stop=(t == NT - 1),
                            )
                with tc.tile_pool(name="ph3f", bufs=2) as hsp3:
                    for c in range(8):
                        hs3 = hsp3.tile([128, 512], f32, tag="hs3")
                        nc.scalar.copy(hs3[:], hb2[c][:])
                        nc.sync.dma_start(hb1_d[:, c, :], hs3[:])

    nc.compile()
    return nc


def _build_p2(trivial_affine: bool):
    bacc, bass, tile, mybir = _bass_mods()
    f32 = mybir.dt.float32
    f32r = mybir.dt.float32r
    AF = mybir.ActivationFunctionType
    OP = mybir.AluOpType
    ts = bass.ts

    nc = bacc.Bacc(
        "TRN2",
        target_bir_lowering=False,
        debug=False,
        enable_asserts=False,
        num_devices=NC_,
    )

    xt_d = nc.dram_tensor("xt", [NT, 128, 8, 128], f32, kind="ExternalInput")
    wst_d = nc.dram_tensor("wst", [128, 8, D], f32, kind="ExternalInput")
    wfn_d = nc.dram_tensor("wfn", [128, 8, D], f32, kind="ExternalInput")
    gb_d = nc.dram_tensor("gb", [128, NT], f32, kind="ExternalInput")
    bb_d = nc.dram_tensor("bb", [128, NT], f32, kind="ExternalInput")
    eps_d = nc.dram_tensor("epsv", [128, 1], f32, kind="ExternalInput")
    if not trivial_affine:
        lng_d = nc.dram_tensor("lng", [128, D], f32, kind="ExternalInput")
        lnb_d = nc.dram_tensor("lnb", [128, D], f32, kind="ExternalInput")
    y_d = nc.dram_tensor("y", [NT, 128, D], f32, kind="ExternalOutput")

    with tile.TileContext(nc) as tc:
        with tc.tile_pool(name="wp", bufs=1) as wp, \
             tc.tile_pool(name="sp", bufs=1) as sp:
            wst_sb = wp.tile([128, 8, D], f32, tag="wst")
            wfn_sb = wp.tile([128, 8, D], f32, tag="wfn")
            nc.sync.dma_start(wst_sb[:].bitcast(f32r), wst_d[:].bitcast(f32r))
            nc.sync.dma_start(wfn_sb[:].bitcast(f32r), wfn_d[:].bitcast(f32r))
            gb_sb = sp.tile([128, NT], f32, tag="gb")
            bb_sb = sp.tile([128, NT], f32, tag="bb")
            eps_b = sp.tile([128, 1], f32, tag="eps")
            nc.sync.dma_start(gb_sb[:], gb_d[:])
            nc.sync.dma_start(bb_sb[:], bb_d[:])
            nc.sync.dma_start(eps_b[:], eps_d[:])
            if not trivial_affine:
                lng_sb = sp.tile([128, D], f32, tag="lng")
                lnb_sb = sp.tile([128, D], f32, tag="lnb")
                nc.sync.dma_start(lng_sb[:], lng_d[:])
                nc.sync.dma_start(lnb_sb[:], lnb_d[:])

            with tc.tile_pool(name="pbx", bufs=3) as pbx, \
                 tc.tile_pool(name="pbu", bufs=1) as pbu, \
                 tc.tile_pool(name="pbp", bufs=4, space=bass.MemorySpace.PSUM) as psb:
                for t in range(NT):
                    xtb = pbx.tile([128, 8, 128], f32, tag="xtb")
                    nc.sync.dma_start(xtb[:].bitcast(f32r), xt_d[t].bitcast(f32r))
                    comb = pbu.tile([128, D], f32, tag="comb", bufs=2)
                    for h in range(2):
                        sps = psb.tile([128, 512], f32, tag="sps")
                        for c in range(8):
                            nc.tensor.matmul(
                                sps[:],
                                xtb[:, c, :].bitcast(f32r),
                                wst_sb[:, c, ts(h, 512)].bitcast(f32r),
                                start=(c == 0),
                                stop=(c == 7),
                            )
                        fps = psb.tile([128, 512], f32, tag="fps")
                        for c in range(8):
                            nc.tensor.matmul(
                                fps[:],
                                xtb[:, c, :].bitcast(f32r),
                                wfn_sb[:, c, ts(h, 512)].bitcast(f32r),
                                start=(c == 0),
                                stop=(c == 7),
                            )
                        nc.scalar.copy(comb[:, ts(h, 512)], sps[:])
                        nc.vector.scalar_tensor_tensor(
                            comb[:, ts(h, 512)], fps[:],
                            gb_sb[:, t:t + 1],
                            comb[:, ts(h, 512)],
                            op0=OP.mult, op1=OP.add,
                        )
                    # pre-scale by beta (row AP), then plain Silu with row sum
                    u0 = pbu.tile([128, D], f32, tag="u0", bufs=2)
                    nc.vector.tensor_scalar_mul(
                        u0[:], comb[:], bb_sb[:, t:t + 1]
                    )
                    u = pbu.tile([128, D], f32, tag="u", bufs=2)
                    usum = pbu.tile([128, 1], f32, tag="usum", bufs=2)
                    nc.scalar.activation(
                        u[:], u0[:], AF.Silu, accum_out=usum[:],
                    )
                    ussq = pbu.tile([128, 1], f32, tag="ussq", bufs=2)
                    nc.scalar.activation(
                        u0[:], u[:], AF.Square, accum_out=ussq[:],
                    )
                    msc = pbu.tile([128, 1], f32, tag="msc", bufs=2)
                    nc.vector.tensor_scalar_mul(msc[:], usum[:], 1.0 / D)
                    msq = pbu.tile([128, 1], f32, tag="msq", bufs=2)
                    nc.vector.tensor_tensor(msq[:], msc[:], msc[:], OP.mult)
                    varv = pbu.tile([128, 1], f32, tag="varv", bufs=2)
                    nc.vector.scalar_tensor_tensor(
                        varv[:], ussq[:], 1.0 / D, msq[:],
                        op0=OP.mult, op1=OP.subtract,
                    )
                    sd = pbu.tile([128, 1], f32, tag="sd", bufs=2)
                    nc.scalar.activation(sd[:], varv[:], AF.Sqrt, bias=eps_b[:])
                    rstd = pbu.tile([128, 1], f32, tag="rstd", bufs=2)
                    nc.vector.reciprocal(rstd[:], sd[:])
                    nbias = pbu.tile([128, 1], f32, tag="nbias", bufs=2)
                    nc.vector.scalar_tensor_tensor(
                        nbias[:], msc[:], -1.0, rstd[:],
                        op0=OP.mult, op1=OP.mult,
                    )
                    nc.vector.tensor_scalar(
                        comb[:], u[:],
                        rstd[:], nbias[:],
                        op0=OP.mult, op1=OP.add,
                    )
                    if not trivial_affine:
                        nc.vector.tensor_tensor(
                            comb[:], comb[:], lng_sb[:], OP.mult
                        )
                        nc.vector.tensor_tensor(
                            comb[:], comb[:], lnb_sb[:], OP.add
                        )
                    nc.sync.dma_start(y_d[t], comb[:])

    nc.compile()
    return nc


def _get_compiled(key):
    if key not in _COMPILED:
        if key == "p1":
            _COMPILED[key] = _build_p1()
        else:
            _COMPILED[key] = _build_p2(key[1])
    return _COMPILED[key]


def _fold_coefs(task_loss, W_slow, w1, b1, g16, bn16, w2, b2):
    A = w1[:, 0].astype(np.float64) / (D - 1)
    Bv = w1[:, 1].astype(np.float64) / D
    w_norm = float(np.sqrt(np.sum(W_slow.astype(np.float64) ** 2)))
    c = (w1[:, 2].astype(np.float64) * w_norm
         + w1[:, 3].astype(np.float64) * task_loss
         + b1.astype(np.float64))
    Ab = A - A.mean()
    Bb = Bv - Bv.mean()
    cb = c - c.mean()
    g = g16.astype(np.float64)
    row = np.zeros(128, np.float64)
    row[0:16] = g * Ab
    row[16:32] = g * Bb
    row[32:48] = g * cb
    row[48:64] = bn16
    row[64:112] = np.asarray(w2, np.float64).reshape(-1)
    row[112:115] = b2
    row[115:121] = [
        (Ab * Ab).mean(),
        (Bb * Bb).mean(),
        2.0 * (Ab * Bb).mean(),
        2.0 * (Ab * cb).mean(),
        2.0 * (Bb * cb).mean(),
        (cb * cb).mean() + EPS,
    ]
    return np.ascontiguousarray(
        np.broadcast_to(row.astype(np.float32), (128, 128))
    )


class _Res:
    def __init__(self, *rs):
        vals = [r.exec_time_ns for r in rs if r.exec_time_ns is not None]
        self.exec_time_ns = sum(vals) if vals else None
        self.parts = rs


def _run(inputs, trace=False):
    # the axon NTFF profile hook works once per process: trace at most one
    # launch. trace may be a bool (applies to launch 1 only) or a pair.
    if isinstance(trace, tuple):
        trace1, trace2 = trace
    else:
        trace1, trace2 = trace, False
    x = np.ascontiguousarray(np.asarray(inputs["x"], dtype=np.float32))
    task_loss = float(np.asarray(inputs["task_loss"]))
    W_slow = np.asarray(inputs["W_slow"], dtype=np.float32)
    W_fast = np.asarray(inputs["W_fast"], dtype=np.float32)
    w1 = np.asarray(inputs["reg_w1"], dtype=np.float32)
    b1 = np.asarray(inputs["reg_b1"], dtype=np.float32)
    g16 = np.asarray(inputs["reg_g"], dtype=np.float32)
    bn16 = np.asarray(inputs["reg_bn"], dtype=np.float32)
    w2 = np.asarray(inputs["reg_w2"], dtype=np.float32)
    b2 = np.asarray(inputs["reg_b2"], dtype=np.float32)
    ln_g = np.asarray(inputs["ln_g"], dtype=np.float32)
    ln_b = np.asarray(inputs["ln_b"], dtype=np.float32)

    trivial = bool(np.all(ln_g == 1.0) and np.all(ln_b == 0.0))
    nc1 = _get_compiled("p1")
    nc2 = _get_compiled(("p2", trivial))

    from concourse.bass_utils import run_bass_kernel_spmd

    wst = np.ascontiguousarray(
        W_slow.T.reshape(8, 128, D).transpose(1, 0, 2)
    )
    coef = _fold_coefs(task_loss, W_slow, w1, b1, g16, bn16, w2, b2)

    xs = x.reshape(NC_, BC, D)
    xts = []
    in_maps1 = []
    for ci in range(NC_):
        xc = xs[ci]
        xt = np.ascontiguousarray(
            xc.T.reshape(8, 128, NT, 128).transpose(2, 1, 0, 3)
        )
        xts.append(xt)
        in_maps1.append({
            "xt": xt,
            "xn": np.ascontiguousarray(xc.reshape(NT, 128, D)),
            "wst": wst,
            "coef": coef,
        })

    res1 = run_bass_kernel_spmd(
        nc1, in_maps1, core_ids=list(range(NC_)), trace=trace1
    )
    outs1 = res1.results

    met = np.empty((B, 1), dtype=np.float32)
    sen = np.empty((B, 1), dtype=np.float32)
    gate = np.empty((B, 1), dtype=np.float32)
    msgs = []
    HT = np.zeros((128, 8, D), dtype=np.float64)
    for ci in range(NC_):
        r = outs1[ci]
        msg = np.asarray(r["msg"])
        msgs.append(msg)
        met[ci * BC:(ci + 1) * BC] = msg[:, 0:32].T.reshape(BC, 1)
        sen[ci * BC:(ci + 1) * BC] = msg[:, 32:64].T.reshape(BC, 1)
        gate[ci * BC:(ci + 1) * BC] = msg[:, 64:96].T.reshape(BC, 1)
        HT[:, :, 0:512] += np.asarray(r["hb0"], dtype=np.float64)
        HT[:, :, 512:1024] += np.asarray(r["hb1"], dtype=np.float64)

    # HT[p, c, j] = (x.T @ slow)[c*128+p, j];  hebb = (slow.T @ x)/B
    xts_full = HT.transpose(1, 0, 2).reshape(D, D)
    hebb = xts_full.T / B
    rate = float(np.mean(met.astype(np.float64))) * 0.1
    W_fast_new = np.asarray(
        (W_fast.astype(np.float64) + rate * np.tanh(hebb)) * 0.9995,
        dtype=np.float32,
    )

    wfnT = np.ascontiguousarray(
        W_fast_new.T.reshape(8, 128, D).transpose(1, 0, 2)
    )
    epsv = np.full((128, 1), EPS, dtype=np.float32)
    in_maps2 = []
    for ci in range(NC_):
        msg = msgs[ci]
        m = {
            "xt": xts[ci],
            "wst": wst,
            "wfn": wfnT,
            "gb": np.ascontiguousarray(msg[:, 64:96]),
            "bb": np.ascontiguousarray(0.5 + 2.0 * msg[:, 32:64]),
            "epsv": epsv,
        }
        if not trivial:
            m["lng"] = np.ascontiguousarray(
                np.broadcast_to(ln_g, (128, D)).astype(np.float32)
            )
            m["lnb"] = np.ascontiguousarray(
                np.broadcast_to(ln_b, (128, D)).astype(np.float32)
            )
        in_maps2.append(m)

    res2 = run_bass_kernel_spmd(
        nc2, in_maps2, core_ids=list(range(NC_)), trace=trace2
    )
    outs2 = res2.results

    y = np.empty((B, D), dtype=np.float32)
    for ci in range(NC_):
        y[ci * BC:(ci + 1) * BC] = np.asarray(outs2[ci]["y"]).reshape(BC, D)

    return (y, met, sen, gate, W_fast_new), _Res(res1, res2)


def kernel(**inputs):
    return _run(inputs, trace=False)[0]
